# revision 5
# baseline (speedup 1.0000x reference)
"""DeepGraphSAGE Trainium2 kernel v3 (8 NeuronCores, data-parallel over graphs).

v3 over v2: all feature-major -> node-major transposes run on the DMA
transpose XBAR (SBUF->SBUF, bf16) instead of the PE, writing the
node-major tiles directly (no PSUM, no copy ops). Node chunks overlap
(starts 0/128/256/272, all 128 wide) so every transpose is XBAR-legal;
the host zeroes adjacency rows for the duplicated span and the top-k
weight chunk gets one memset. PSUM rings are deeper (uv x2, az x4,
azS x2) for cross-graph pipelining.
"""

import sys

sys.path.insert(0, "/opt/trn_rl_repo")

import numpy as np
import ml_dtypes

import concourse.bass as bass
import concourse.bacc as bacc
import concourse.mybir as mybir
from concourse.tile import TileContext
from concourse.bass_utils import run_bass_kernel_spmd

BF16 = ml_dtypes.bfloat16
F32 = mybir.dt.float32
B16 = mybir.dt.bfloat16

NCORES = 8
B = 512          # graphs
NPG = 400        # nodes per graph (= free dim, no padding)
EPG = 6400       # edges per graph
F_IN = 200       # input feature dim
H = 64           # hidden
G = B // NCORES  # graphs per core
CS = (0, 128, 256, 272)   # overlapping node chunk starts, all 128 wide
KTOP = NPG // 2  # top-k = 200

AX = mybir.AxisListType.X
OP = mybir.AluOpType
AF = mybir.ActivationFunctionType


# ----------------------------------------------------------------------------
# Device kernel
# ----------------------------------------------------------------------------

def build_kernel(gc=G, bpr_val=0.0, n_bisect=26, dbg=False):
    nc = bacc.Bacc("TRN2", debug=False)
    D = NPG

    xa_d = nc.declare_dram_parameter("xa", [gc, 128, D], B16, isOutput=False)
    xb_d = nc.declare_dram_parameter("xb", [gc, 72, D], B16, isOutput=False)
    adj_d = nc.declare_dram_parameter("adj", [gc, 128, 4 * D], B16, isOutput=False)
    deg_d = nc.declare_dram_parameter("deg", [128, gc * 4], F32, isOutput=False)
    cw_d = nc.declare_dram_parameter("cw16", [128, 520], B16, isOutput=False)
    cf_d = nc.declare_dram_parameter("cf32", [128, 136], F32, isOutput=False)
    out_d = nc.declare_dram_parameter("out", [2, gc], F32, isOutput=True)
    if dbg:
        dbg_uv = nc.declare_dram_parameter("dbg_uv", [64, D], B16, isOutput=True)
        dbg_h = nc.declare_dram_parameter("dbg_h", [3, 64, D], B16, isOutput=True)
        dbg_S = nc.declare_dram_parameter("dbg_S", [gc, D], F32, isOutput=True)
        dbg_lo = nc.declare_dram_parameter("dbg_lo", [gc, 1], F32, isOutput=True)
        dbg_w = nc.declare_dram_parameter("dbg_w", [gc, D], F32, isOutput=True)
        dbg_pool = nc.declare_dram_parameter("dbg_pool", [65, gc], F32, isOutput=True)

    with TileContext(nc) as tc:
        with (
            tc.tile_pool(name="const", bufs=1) as cpool,
            tc.tile_pool(name="xp", bufs=4) as xpool,
            tc.tile_pool(name="ap", bufs=4) as apool,
            tc.tile_pool(name="up", bufs=3) as upool,
            tc.tile_pool(name="hp", bufs=9) as hpool,
            tc.tile_pool(name="np", bufs=9) as npool,
            tc.tile_pool(name="kp", bufs=gc) as kpool,
            tc.tile_pool(name="puv", bufs=2, space="PSUM") as puv,
            tc.tile_pool(name="paz", bufs=4, space="PSUM") as paz,
            tc.tile_pool(name="pas", bufs=2, space="PSUM") as pas,
        ):
            # ---- constants ----
            cw = cpool.tile([128, 520], B16, tag="cw")
            nc.sync.dma_start(out=cw[:], in_=cw_d[:])
            cf = cpool.tile([128, 136], F32, tag="cf")
            nc.sync.dma_start(out=cf[:], in_=cf_d[:])
            deg_sb = cpool.tile([128, gc * 4], F32, tag="deg")
            nc.sync.dma_start(out=deg_sb[:], in_=deg_d[:])

            w1a = cw[:, 128:256]            # W1cat rows 0:128   [128,128]
            w1b = cw[0:72, 256:384]         # W1cat rows 128:200 [72,128]
            w2 = cw[:, 384:448]             # [W2r; W2l] [128,64]
            w3 = cw[:, 448:512]             # [W3r; W3l] [128,64]
            wsc = cw[:, 512:514]            # [128,2]: col0=[Wpo;0], col1=[0;Wpr]
            wlin = cw[0:65, 514:516]        # [Wlin; blin] [65,2]
            identf = cf[:, 0:128]           # f32 I128
            b1hi = cf[64:128, 128:129]      # biases live at partitions 64:128:
            b2hi = cf[64:128, 129:130]      # relu inputs sit in psum rows 64:128
            b3hi = cf[64:128, 130:131]

            # node-major raw scores: s_all[p, c*gc + g]
            s_all = cpool.tile([128, 4 * gc], F32, tag="sall")

            h3keep = []

            # ------------------------- main graph loop -------------------------
            for g in range(gc):
                xa_t = xpool.tile([128, D], B16, tag="xa")
                nc.sync.dma_start(out=xa_t[:], in_=xa_d[g])
                xb_t = xpool.tile([72, D], B16, tag="xb")
                nc.sync.dma_start(out=xb_t[:], in_=xb_d[g])
                a_t = apool.tile([128, 4 * D], B16, tag="a")
                nc.sync.dma_start(out=a_t[:], in_=adj_d[g])

                def a_chunk(c):
                    return a_t[:, c * D:(c + 1) * D]

                # ---- L1 linear: uv[j, n] = (x @ [W1l|W1r])^T ----
                uv_ps = puv.tile([128, D], F32, tag="uv")
                nc.tensor.matmul(uv_ps[:], w1a, xa_t[:], start=True, stop=False)
                nc.tensor.matmul(uv_ps[:], w1b, xb_t[:], start=False, stop=True)
                u_sb = upool.tile([64, D], B16, tag="u")
                nc.scalar.activation(u_sb[:], uv_ps[0:64, :], AF.Copy)
                if dbg and g == 0:
                    nc.sync.dma_start(out=dbg_uv[:], in_=u_sb[:])

                # transpose u -> node-major [128, 4, 64] via DMA XBAR
                u_nm = npool.tile([128, 4, 64], B16, tag="nm")
                for c in range(4):
                    nc.sync.dma_start(out=u_nm[:, c, :],
                                      in_=u_sb[:, CS[c]:CS[c] + 128],
                                      transpose=True)

                # ---- L1 aggregation accumulates onto the v half of uv_ps ----
                for c in range(4):
                    nc.tensor.matmul(uv_ps[64:128, :], u_nm[:, c, :],
                                     a_chunk(c), start=False, stop=(c == 3),
                                     skip_group_check=True)
                hst1 = hpool.tile([128, D], B16, tag="h")
                nc.scalar.activation(hst1[0:64, :], uv_ps[64:128, :], AF.Relu,
                                     bias=b1hi)
                if dbg and g == 0:
                    nc.sync.dma_start(out=dbg_h[0], in_=hst1[0:64, :])

                # ---- layers 2 and 3 ----
                hprev = hst1
                for li, (wcat, bb) in enumerate(((w2, b2hi), (w3, b3hi))):
                    # transpose h_prev -> node-major via DMA XBAR
                    h_nm = npool.tile([128, 4, 64], B16, tag="nm")
                    for c in range(4):
                        nc.sync.dma_start(out=h_nm[:, c, :],
                                          in_=hprev[0:64, CS[c]:CS[c] + 128],
                                          transpose=True)
                    # aggregation (A_norm moving); z shares the tile's
                    # upper partition half (same psum bank, rows 64:128)
                    az = paz.tile([128, 408], F32, tag="az")
                    for c in range(4):
                        nc.tensor.matmul(az[0:64, 0:D], h_nm[:, c, :],
                                         a_chunk(c), start=(c == 0),
                                         stop=(c == 3))
                    # stack agg under h (bf16)
                    nc.vector.tensor_copy(hprev[64:128, :], az[0:64, 0:D])
                    # z = [h; agg] @ [Wr; Wl]
                    nc.tensor.matmul(az[64:128, 0:D], wcat, hprev[:],
                                     start=True, stop=True)
                    # h_next = relu(z + b) + h_prev
                    hn = hpool.tile([128, D], B16, tag="h")
                    nc.scalar.activation(hn[0:64, :], az[64:128, 0:D], AF.Relu,
                                         bias=bb)
                    nc.gpsimd.tensor_tensor(hn[0:64, :], hn[0:64, :],
                                            hprev[0:64, :], OP.add)
                    if dbg and g == 0:
                        nc.sync.dma_start(out=dbg_h[1 + li], in_=hn[0:64, :])
                    hprev = hn

                hst3 = hprev
                # transpose h3 -> node-major (kept for score-agg + pooling)
                h3_nm = kpool.tile([128, 4, 64], B16, tag="h3k")
                for c in range(4):
                    nc.sync.dma_start(out=h3_nm[:, c, :],
                                      in_=hst3[0:64, CS[c]:CS[c] + 128],
                                      transpose=True)
                h3keep.append(h3_nm)

                # ---- score aggregation (normalized; deg folded in later);
                # snm projections live in the same psum tile, cols 400:408 ----
                azS = pas.tile([128, 408], F32, tag="as")
                for c in range(4):
                    nc.tensor.matmul(azS[0:64, 0:D], h3_nm[:, c, :],
                                     a_chunk(c), start=(c == 0), stop=(c == 3))
                nc.vector.tensor_copy(hst3[64:128, :], azS[0:64, 0:D])

                # node-major projections: [sB, sA] per node chunk
                for c in range(4):
                    nc.tensor.matmul(azS[:, D + 2 * c:D + 2 * c + 2],
                                     hst3[:, CS[c]:CS[c] + 128], wsc,
                                     start=True, stop=True)
                # s = sB + deg * sA + bpr   (node-major, strided into s_all)
                tmp4 = cpool.tile([128, 4], F32, tag="tmp4")
                nc.vector.tensor_tensor(tmp4[:], azS[:, D + 1:D + 8:2],
                                        deg_sb[:, 4 * g:4 * g + 4], OP.mult)
                nc.vector.scalar_tensor_tensor(
                    s_all[:, g:4 * gc:gc], tmp4[:], float(bpr_val),
                    azS[:, D:D + 7:2], OP.add, OP.add)

            # ------------------------- top-k threshold -------------------------
            # regroup node-major scores into graph-major S[g, c*128+p]
            # (chunk 3 holds nodes 272:400; keep only its last 16 columns)
            S = cpool.tile([gc, 512], F32, tag="S")
            for c in range(4):
                stp_ps = pas.tile([gc, 128], F32, tag="as")
                nc.tensor.transpose(stp_ps[:], s_all[:, c * gc:(c + 1) * gc],
                                    identf)
                if c < 3:
                    nc.vector.tensor_copy(S[:, c * 128:(c + 1) * 128], stp_ps[:])
                else:
                    nc.vector.tensor_copy(S[:, 384:400], stp_ps[:, 112:128])
            if dbg:
                nc.sync.dma_start(out=dbg_S[:], in_=S[:, 0:D])

            S2 = cpool.tile([gc, D], F32, tag="S2")
            nc.vector.tensor_scalar(S2[:], S[:, 0:D], 2.0, None, OP.mult)
            cmp_s = cpool.tile([gc, D], F32, tag="cmps")
            lo = cpool.tile([gc, 1], F32, tag="lo")
            hi = cpool.tile([gc, 1], F32, tag="hi")
            lohi = cpool.tile([gc, 1], F32, tag="lohi")
            mid = cpool.tile([gc, 1], F32, tag="mid")
            cnt = cpool.tile([gc, 1], F32, tag="cnt")
            msk = cpool.tile([gc, 1], mybir.dt.uint8, tag="msk")
            msk2 = cpool.tile([gc, 1], mybir.dt.uint8, tag="msk2")
            nc.vector.tensor_reduce(lo[:], S[:, 0:D], AX, OP.min)
            nc.vector.tensor_scalar(lo[:], lo[:], -1.0, None, OP.add)
            nc.vector.tensor_reduce(hi[:], S[:, 0:D], AX, OP.max)
            nc.vector.tensor_scalar(hi[:], hi[:], 1.0, None, OP.add)
            for _ in range(n_bisect):
                nc.vector.tensor_tensor(lohi[:], lo[:], hi[:], OP.add)
                nc.vector.tensor_scalar(mid[:], lohi[:], 0.5, None, OP.mult)
                nc.vector.tensor_scalar(cmp_s[:], S2[:], lohi[:], None,
                                        OP.is_ge, OP.add, accum_out=cnt[:])
                nc.vector.tensor_scalar(msk[:], cnt[:], float(KTOP), None,
                                        OP.is_ge)
                nc.vector.tensor_scalar(msk2[:], cnt[:], float(KTOP), None,
                                        OP.is_lt)
                nc.vector.select(lo[:], msk[:], mid[:], lo[:])
                nc.vector.select(hi[:], msk2[:], mid[:], hi[:])
            if dbg:
                nc.sync.dma_start(out=dbg_lo[:], in_=lo[:])

            # w = tanh(s) * (s >= thresh)   (graph-major, 64-row tile so the
            # XBAR transposes below see a full [64, 128] input)
            tnh = cpool.tile([gc, D], F32, tag="tnh")
            nc.scalar.activation(tnh[:], S[:, 0:D], AF.Tanh)
            wgm = cpool.tile([64, D], B16, tag="wgm")
            if gc < 64:
                nc.vector.memset(wgm[:], 0.0)
            nc.vector.scalar_tensor_tensor(
                wgm[0:gc, :], S[:, 0:D], lo[:], tnh[:], OP.is_ge, OP.mult)
            if dbg:
                nc.sync.dma_start(out=dbg_w[:], in_=tnh[:])
            w_nm = cpool.tile([128, 4, 64], B16, tag="wall")
            for c in range(4):
                nc.sync.dma_start(out=w_nm[:, c, :],
                                  in_=wgm[:, CS[c]:CS[c] + 128],
                                  transpose=True)
            # zero the duplicated node span (272:384) in chunk 3
            nc.vector.memset(w_nm[0:112, 3, :], 0.0)

            # ------------------------- pooling + classifier --------------------
            pooled_ps = pas.tile([64, gc], F32, tag="as")
            for g in range(gc):
                for c in range(4):
                    nc.tensor.matmul(pooled_ps[:, g:g + 1],
                                     h3keep[g][:, c, :],
                                     w_nm[:, c, g:g + 1],
                                     start=(c == 0), stop=(c == 3))
            pool_fm = cpool.tile([65, gc], B16, tag="poolfm")
            nc.vector.memset(pool_fm[64:65, :], 1.0)
            nc.scalar.activation(pool_fm[0:64, :], pooled_ps[:], AF.Copy,
                                 scale=1.0 / float(KTOP))
            if dbg:
                pfm_f = cpool.tile([65, gc], F32, tag="pfmf")
                nc.vector.tensor_copy(pfm_f[:], pool_fm[:])
                nc.sync.dma_start(out=dbg_pool[:], in_=pfm_f[:])
            plw = pas.tile([1, 2 * gc], F32, tag="as")
            for cls in range(2):
                nc.tensor.matmul(plw[0:1, cls * gc:(cls + 1) * gc],
                                 wlin[:, cls:cls + 1], pool_fm[:],
                                 start=True, stop=True)
            lgw = cpool.tile([1, 2 * gc], F32, tag="lgw")
            nc.vector.tensor_copy(lgw[:], plw[:])
            m01 = cpool.tile([1, gc], F32, tag="m01")
            d0 = cpool.tile([1, gc], F32, tag="d0")
            d1 = cpool.tile([1, gc], F32, tag="d1")
            e0 = cpool.tile([1, gc], F32, tag="e0")
            e1 = cpool.tile([1, gc], F32, tag="e1")
            lse = cpool.tile([1, gc], F32, tag="lse")
            out_sb = cpool.tile([1, 2 * gc], F32, tag="outsb")
            nc.vector.tensor_tensor(m01[:], lgw[:, 0:gc], lgw[:, gc:2 * gc],
                                    OP.max)
            nc.vector.tensor_tensor(d0[:], lgw[:, 0:gc], m01[:], OP.subtract)
            nc.vector.tensor_tensor(d1[:], lgw[:, gc:2 * gc], m01[:],
                                    OP.subtract)
            nc.scalar.activation(e0[:], d0[:], AF.Exp)
            nc.scalar.activation(e1[:], d1[:], AF.Exp)
            nc.vector.tensor_tensor(lse[:], e0[:], e1[:], OP.add)
            nc.scalar.activation(lse[:], lse[:], AF.Ln)
            nc.vector.tensor_tensor(out_sb[:, 0:gc], d0[:], lse[:], OP.subtract)
            nc.vector.tensor_tensor(out_sb[:, gc:2 * gc], d1[:], lse[:],
                                    OP.subtract)
            ov = out_sb[:].rearrange("p (a b) -> p a b", a=2)[:, :, 0:gc]
            nc.sync.dma_start(out=out_d[:], in_=ov)

    nc.compile()
    return nc


# ----------------------------------------------------------------------------
# Host-side shard/layout prep
# ----------------------------------------------------------------------------

def _prep(x, edge_index, W1l, W1r, b1, W2l, W2r, b2, W3l, W3r, b3,
          Wpr, bpr, Wpo, Wlin, blin, n_graphs=B):
    src = np.asarray(edge_index[0]).astype(np.int64) % NPG
    dst = np.asarray(edge_index[1]).astype(np.int64) % NPG
    key = (src * NPG + dst).reshape(n_graphs, EPG)

    A = np.zeros((n_graphs, NPG * NPG), np.float32)
    for g in range(n_graphs):
        A[g] = np.bincount(key[g], minlength=NPG * NPG)
    A = A.reshape(n_graphs, NPG, NPG)          # A[g, s, d] = edge count s->d
    deg = A.sum(axis=1)                        # in-degree per dst [g, 400]
    inv = 1.0 / np.maximum(deg, 1.0)
    An = A * inv[:, None, :]                   # column-normalized

    # overlapping src chunks (starts 0/128/256/272); chunk 3 keeps only
    # src rows 384:400 (its first 112 rows duplicate chunk 2 -> zeroed)
    adj = np.zeros((n_graphs, 4, 128, NPG), np.float32)
    for c in range(3):
        adj[:, c] = An[:, CS[c]:CS[c] + 128, :]
    adj[:, 3, 112:128, :] = An[:, 384:400, :]
    adj = np.ascontiguousarray(
        adj.transpose(0, 2, 1, 3).reshape(n_graphs, 128, 4 * NPG)).astype(BF16)

    # deg_nm[p, 4g + c] = deg[g, CS[c] + p]
    deg_nm = np.zeros((128, n_graphs, 4), np.float32)
    for c in range(4):
        deg_nm[:, :, c] = deg[:, CS[c]:CS[c] + 128].T
    deg_nm = np.ascontiguousarray(deg_nm.reshape(128, n_graphs * 4))

    x = np.asarray(x, np.float32)
    xT = x.reshape(n_graphs, NPG, F_IN).transpose(0, 2, 1)   # [g, 200, 400]
    xa = np.ascontiguousarray(xT[:, 0:128, :]).astype(BF16)
    xb = np.ascontiguousarray(xT[:, 128:200, :]).astype(BF16)

    def n_(a):
        return np.asarray(a, np.float32)

    cw16 = np.zeros((128, 520), np.float32)
    w1cat = np.concatenate([n_(W1l), n_(W1r)], axis=1)       # [200, 128]
    cw16[:, 128:256] = w1cat[0:128]
    cw16[0:72, 256:384] = w1cat[128:200]
    cw16[:, 384:448] = np.concatenate([n_(W2r), n_(W2l)], axis=0)
    cw16[:, 448:512] = np.concatenate([n_(W3r), n_(W3l)], axis=0)
    cw16[0:64, 512:513] = n_(Wpo)
    cw16[64:128, 513:514] = n_(Wpr)
    cw16[0:64, 514:516] = n_(Wlin)
    cw16[64, 514:516] = n_(blin)
    cw16 = cw16.astype(BF16)

    cf32 = np.zeros((128, 136), np.float32)
    cf32[:, 0:128] = np.eye(128)
    cf32[64:128, 128] = n_(b1)     # biases read at partition base 64
    cf32[64:128, 129] = n_(b2)
    cf32[64:128, 130] = n_(b3)

    return xa, xb, adj, deg_nm, cw16, cf32, float(np.asarray(bpr).reshape(-1)[0])


def kernel(**inputs):
    x = inputs["x"]
    edge_index = inputs["edge_index"]
    xa, xb, adj, deg_nm, cw16, cf32, bpr_val = _prep(
        x, edge_index, inputs["W1l"], inputs["W1r"], inputs["b1"],
        inputs["W2l"], inputs["W2r"], inputs["b2"],
        inputs["W3l"], inputs["W3r"], inputs["b3"],
        inputs["Wpr"], inputs["bpr"], inputs["Wpo"],
        inputs["Wlin"], inputs["blin"])

    nc = build_kernel(G, bpr_val)

    in_maps = []
    for c in range(NCORES):
        gs = slice(c * G, (c + 1) * G)
        in_maps.append({
            "xa": np.ascontiguousarray(xa[gs]),
            "xb": np.ascontiguousarray(xb[gs]),
            "adj": np.ascontiguousarray(adj[gs]),
            "deg": np.ascontiguousarray(
                deg_nm[:, c * G * 4:(c + 1) * G * 4]),
            "cw16": cw16,
            "cf32": cf32,
        })
    res = run_bass_kernel_spmd(nc, in_maps, list(range(NCORES)))
    outs = [res.results[i]["out"] for i in range(NCORES)]    # each [2, G]
    logits = np.concatenate(outs, axis=1).T                  # [512, 2]
    return np.ascontiguousarray(logits.astype(np.float32))


# revision 6
# speedup vs baseline: 2.3577x; 2.3577x over previous
"""DeepGraphSAGE Trainium2 kernel v3 (8 NeuronCores, data-parallel over graphs).

v4: overlapping node chunks (starts 0/128/256/272, all 128 wide) keep
every transpose/matmul uniform; the host zeroes adjacency rows for the
duplicated span and the top-k weight chunk gets one memset. Transposes
run on the PE. Every per-graph PSUM tile has its own single-buffer pool
(8 pools = 8 banks, one alloc per graph each), so a ring slot frees
shortly after use and the Tile scheduler can pipeline 2-3 graphs deep.
"""

import sys

sys.path.insert(0, "/opt/trn_rl_repo")

import numpy as np
import ml_dtypes

import concourse.bass as bass
import concourse.bacc as bacc
import concourse.mybir as mybir
from concourse.tile import TileContext
from concourse.bass_utils import run_bass_kernel_spmd

BF16 = ml_dtypes.bfloat16
F32 = mybir.dt.float32
B16 = mybir.dt.bfloat16

NCORES = 8
B = 512          # graphs
NPG = 400        # nodes per graph (= free dim, no padding)
EPG = 6400       # edges per graph
F_IN = 200       # input feature dim
H = 64           # hidden
G = B // NCORES  # graphs per core
CS = (0, 128, 256, 272)   # overlapping node chunk starts, all 128 wide
KTOP = NPG // 2  # top-k = 200

AX = mybir.AxisListType.X
OP = mybir.AluOpType
AF = mybir.ActivationFunctionType


# ----------------------------------------------------------------------------
# Device kernel
# ----------------------------------------------------------------------------

def build_kernel(gc=G, bpr_val=0.0, n_bisect=26, dbg=False):
    nc = bacc.Bacc("TRN2", debug=False)
    D = NPG

    xa_d = nc.declare_dram_parameter("xa", [gc, 128, D], B16, isOutput=False)
    xb_d = nc.declare_dram_parameter("xb", [gc, 72, D], B16, isOutput=False)
    adj_d = nc.declare_dram_parameter("adj", [gc, 128, 4 * D], B16, isOutput=False)
    deg_d = nc.declare_dram_parameter("deg", [128, gc * 4], F32, isOutput=False)
    cw_d = nc.declare_dram_parameter("cw16", [128, 520], B16, isOutput=False)
    cf_d = nc.declare_dram_parameter("cf32", [128, 136], F32, isOutput=False)
    out_d = nc.declare_dram_parameter("out", [2, gc], F32, isOutput=True)
    if dbg:
        dbg_uv = nc.declare_dram_parameter("dbg_uv", [64, D], B16, isOutput=True)
        dbg_h = nc.declare_dram_parameter("dbg_h", [3, 64, D], B16, isOutput=True)
        dbg_S = nc.declare_dram_parameter("dbg_S", [gc, D], F32, isOutput=True)
        dbg_lo = nc.declare_dram_parameter("dbg_lo", [gc, 1], F32, isOutput=True)
        dbg_w = nc.declare_dram_parameter("dbg_w", [gc, D], F32, isOutput=True)
        dbg_pool = nc.declare_dram_parameter("dbg_pool", [65, gc], F32, isOutput=True)

    with TileContext(nc) as tc:
        with (
            tc.tile_pool(name="const", bufs=1) as cpool,
            tc.tile_pool(name="xp", bufs=4) as xpool,
            tc.tile_pool(name="ap", bufs=4) as apool,
            tc.tile_pool(name="up", bufs=3) as upool,
            tc.tile_pool(name="hp", bufs=9) as hpool,
            tc.tile_pool(name="np", bufs=9) as npool,
            tc.tile_pool(name="kp", bufs=gc) as kpool,
            tc.tile_pool(name="puv", bufs=1, space="PSUM") as puv,
            tc.tile_pool(name="paz2", bufs=1, space="PSUM") as paz2,
            tc.tile_pool(name="paz3", bufs=1, space="PSUM") as paz3,
            tc.tile_pool(name="pas", bufs=1, space="PSUM") as pas,
            tc.tile_pool(name="ptp0", bufs=1, space="PSUM") as ptp0,
            tc.tile_pool(name="ptp1", bufs=1, space="PSUM") as ptp1,
            tc.tile_pool(name="ptp2", bufs=1, space="PSUM") as ptp2,
            tc.tile_pool(name="ptp3", bufs=1, space="PSUM") as ptp3,
        ):
            # ---- constants ----
            cw = cpool.tile([128, 520], B16, tag="cw")
            nc.sync.dma_start(out=cw[:], in_=cw_d[:])
            cf = cpool.tile([128, 136], F32, tag="cf")
            nc.sync.dma_start(out=cf[:], in_=cf_d[:])
            deg_sb = cpool.tile([128, gc * 4], F32, tag="deg")
            nc.sync.dma_start(out=deg_sb[:], in_=deg_d[:])

            id64 = cw[0:64, 0:64]           # bf16 I64 (PE transposes)
            w1a = cw[:, 128:256]            # W1cat rows 0:128   [128,128]
            w1b = cw[0:72, 256:384]         # W1cat rows 128:200 [72,128]
            w2 = cw[:, 384:448]             # [W2r; W2l] [128,64]
            w3 = cw[:, 448:512]             # [W3r; W3l] [128,64]
            wsc = cw[:, 512:514]            # [128,2]: col0=[Wpo;0], col1=[0;Wpr]
            wlin = cw[0:65, 514:516]        # [Wlin; blin] [65,2]
            identf = cf[:, 0:128]           # f32 I128
            b1hi = cf[64:128, 128:129]      # biases live at partitions 64:128:
            b2hi = cf[64:128, 129:130]      # relu inputs sit in psum rows 64:128
            b3hi = cf[64:128, 130:131]

            # node-major raw scores: s_all[p, c*gc + g]
            s_all = cpool.tile([128, 4 * gc], F32, tag="sall")

            h3keep = []

            # ------------------------- main graph loop -------------------------
            for g in range(gc):
                xa_t = xpool.tile([128, D], B16, tag="xa")
                nc.sync.dma_start(out=xa_t[:], in_=xa_d[g])
                xb_t = xpool.tile([72, D], B16, tag="xb")
                nc.sync.dma_start(out=xb_t[:], in_=xb_d[g])
                a_t = apool.tile([128, 4 * D], B16, tag="a")
                nc.sync.dma_start(out=a_t[:], in_=adj_d[g])

                def a_chunk(c):
                    return a_t[:, c * D:(c + 1) * D]

                # ---- L1 linear: uv[j, n] = (x @ [W1l|W1r])^T ----
                uv_ps = puv.tile([128, D], F32, tag="uv")
                nc.tensor.matmul(uv_ps[:], w1a, xa_t[:], start=True, stop=False)
                nc.tensor.matmul(uv_ps[:], w1b, xb_t[:], start=False, stop=True)
                u_sb = upool.tile([64, D], B16, tag="u")
                nc.scalar.activation(u_sb[:], uv_ps[0:64, :], AF.Copy)
                if dbg and g == 0:
                    nc.sync.dma_start(out=dbg_uv[:], in_=u_sb[:])

                # transpose u -> node-major [128, 4, 64] on the PE
                u_tp = ptp0.tile([128, 4, 64], B16, tag="tp")
                for c in range(4):
                    nc.tensor.transpose(u_tp[:, c, :],
                                        u_sb[:, CS[c]:CS[c] + 128], id64)
                u_nm = npool.tile([128, 4, 64], B16, tag="nm")
                nc.vector.tensor_copy(u_nm[:], u_tp[:])

                # ---- L1 aggregation accumulates onto the v half of uv_ps ----
                for c in range(4):
                    nc.tensor.matmul(uv_ps[64:128, :], u_nm[:, c, :],
                                     a_chunk(c), start=False, stop=(c == 3),
                                     skip_group_check=True)
                hst1 = hpool.tile([128, D], B16, tag="h")
                nc.scalar.activation(hst1[0:64, :], uv_ps[64:128, :], AF.Relu,
                                     bias=b1hi)
                if dbg and g == 0:
                    nc.sync.dma_start(out=dbg_h[0], in_=hst1[0:64, :])

                # ---- layers 2 and 3 ----
                hprev = hst1
                for li, (wcat, bb, ptpl, pazl) in enumerate(
                        ((w2, b2hi, ptp1, paz2), (w3, b3hi, ptp2, paz3))):
                    # transpose h_prev -> node-major on the PE
                    h_tp = ptpl.tile([128, 4, 64], B16, tag="tp")
                    for c in range(4):
                        nc.tensor.transpose(h_tp[:, c, :],
                                            hprev[0:64, CS[c]:CS[c] + 128],
                                            id64)
                    h_nm = npool.tile([128, 4, 64], B16, tag="nm")
                    nc.vector.tensor_copy(h_nm[:], h_tp[:])
                    # aggregation (A_norm moving); z shares the tile's
                    # upper partition half (same psum bank, rows 64:128)
                    az = pazl.tile([128, 408], F32, tag="az")
                    for c in range(4):
                        nc.tensor.matmul(az[0:64, 0:D], h_nm[:, c, :],
                                         a_chunk(c), start=(c == 0),
                                         stop=(c == 3))
                    # stack agg under h (bf16)
                    if li == 0:
                        nc.scalar.activation(hprev[64:128, :], az[0:64, 0:D],
                                             AF.Copy)
                    else:
                        nc.vector.tensor_copy(hprev[64:128, :], az[0:64, 0:D])
                    # z = [h; agg] @ [Wr; Wl]
                    nc.tensor.matmul(az[64:128, 0:D], wcat, hprev[:],
                                     start=True, stop=True)
                    # h_next = relu(z + b) + h_prev
                    hn = hpool.tile([128, D], B16, tag="h")
                    nc.scalar.activation(hn[0:64, :], az[64:128, 0:D], AF.Relu,
                                         bias=bb)
                    nc.gpsimd.tensor_tensor(hn[0:64, :], hn[0:64, :],
                                            hprev[0:64, :], OP.add)
                    if dbg and g == 0:
                        nc.sync.dma_start(out=dbg_h[1 + li], in_=hn[0:64, :])
                    hprev = hn

                hst3 = hprev
                # transpose h3 -> node-major (kept for score-agg + pooling)
                h3_tp = ptp3.tile([128, 4, 64], B16, tag="tp")
                for c in range(4):
                    nc.tensor.transpose(h3_tp[:, c, :],
                                        hst3[0:64, CS[c]:CS[c] + 128], id64)
                h3_nm = kpool.tile([128, 4, 64], B16, tag="h3k")
                nc.vector.tensor_copy(h3_nm[:], h3_tp[:])
                h3keep.append(h3_nm)

                # ---- score aggregation (normalized; deg folded in later);
                # snm projections live in the same psum tile, cols 400:408 ----
                azS = pas.tile([128, 408], F32, tag="as")
                for c in range(4):
                    nc.tensor.matmul(azS[0:64, 0:D], h3_nm[:, c, :],
                                     a_chunk(c), start=(c == 0), stop=(c == 3))
                nc.vector.tensor_copy(hst3[64:128, :], azS[0:64, 0:D])

                # node-major projections: [sB, sA] per node chunk
                for c in range(4):
                    nc.tensor.matmul(azS[:, D + 2 * c:D + 2 * c + 2],
                                     hst3[:, CS[c]:CS[c] + 128], wsc,
                                     start=True, stop=True)
                # s = sB + deg * sA + bpr   (node-major, strided into s_all)
                tmp4 = cpool.tile([128, 4], F32, tag="tmp4")
                nc.vector.tensor_tensor(tmp4[:], azS[:, D + 1:D + 8:2],
                                        deg_sb[:, 4 * g:4 * g + 4], OP.mult)
                nc.vector.scalar_tensor_tensor(
                    s_all[:, g:4 * gc:gc], tmp4[:], float(bpr_val),
                    azS[:, D:D + 7:2], OP.add, OP.add)

            # ------------------------- top-k threshold -------------------------
            # regroup node-major scores into graph-major S[g, c*128+p]
            # (chunk 3 holds nodes 272:400; keep only its last 16 columns)
            S = cpool.tile([gc, 512], F32, tag="S")
            for c in range(4):
                stp_ps = pas.tile([gc, 128], F32, tag="as")
                nc.tensor.transpose(stp_ps[:], s_all[:, c * gc:(c + 1) * gc],
                                    identf)
                if c < 3:
                    nc.vector.tensor_copy(S[:, c * 128:(c + 1) * 128], stp_ps[:])
                else:
                    nc.vector.tensor_copy(S[:, 384:400], stp_ps[:, 112:128])
            if dbg:
                nc.sync.dma_start(out=dbg_S[:], in_=S[:, 0:D])

            S2 = cpool.tile([gc, D], F32, tag="S2")
            nc.vector.tensor_scalar(S2[:], S[:, 0:D], 2.0, None, OP.mult)
            cmp_s = cpool.tile([gc, D], F32, tag="cmps")
            lo = cpool.tile([gc, 1], F32, tag="lo")
            hi = cpool.tile([gc, 1], F32, tag="hi")
            lohi = cpool.tile([gc, 1], F32, tag="lohi")
            mid = cpool.tile([gc, 1], F32, tag="mid")
            cnt = cpool.tile([gc, 1], F32, tag="cnt")
            msk = cpool.tile([gc, 1], mybir.dt.uint8, tag="msk")
            msk2 = cpool.tile([gc, 1], mybir.dt.uint8, tag="msk2")
            nc.vector.tensor_reduce(lo[:], S[:, 0:D], AX, OP.min)
            nc.vector.tensor_scalar(lo[:], lo[:], -1.0, None, OP.add)
            nc.vector.tensor_reduce(hi[:], S[:, 0:D], AX, OP.max)
            nc.vector.tensor_scalar(hi[:], hi[:], 1.0, None, OP.add)
            for _ in range(n_bisect):
                nc.vector.tensor_tensor(lohi[:], lo[:], hi[:], OP.add)
                nc.vector.tensor_scalar(mid[:], lohi[:], 0.5, None, OP.mult)
                nc.vector.tensor_scalar(cmp_s[:], S2[:], lohi[:], None,
                                        OP.is_ge, OP.add, accum_out=cnt[:])
                nc.vector.tensor_scalar(msk[:], cnt[:], float(KTOP), None,
                                        OP.is_ge)
                nc.vector.tensor_scalar(msk2[:], cnt[:], float(KTOP), None,
                                        OP.is_lt)
                nc.vector.select(lo[:], msk[:], mid[:], lo[:])
                nc.vector.select(hi[:], msk2[:], mid[:], hi[:])
            if dbg:
                nc.sync.dma_start(out=dbg_lo[:], in_=lo[:])

            # w = tanh(s) * (s >= thresh)   (graph-major, 64-row tile so the
            # XBAR transposes below see a full [64, 128] input)
            tnh = cpool.tile([gc, D], F32, tag="tnh")
            nc.scalar.activation(tnh[:], S[:, 0:D], AF.Tanh)
            wgm = cpool.tile([64, D], B16, tag="wgm")
            if gc < 64:
                nc.vector.memset(wgm[:], 0.0)
            nc.vector.scalar_tensor_tensor(
                wgm[0:gc, :], S[:, 0:D], lo[:], tnh[:], OP.is_ge, OP.mult)
            if dbg:
                nc.sync.dma_start(out=dbg_w[:], in_=tnh[:])
            w_tp = ptp0.tile([128, 4, 64], B16, tag="tp")
            for c in range(4):
                nc.tensor.transpose(w_tp[:, c, :],
                                    wgm[0:64, CS[c]:CS[c] + 128],
                                    id64)
            w_nm = cpool.tile([128, 4, 64], B16, tag="wall")
            nc.vector.tensor_copy(w_nm[:], w_tp[:])
            # zero the duplicated node span (272:384) in chunk 3
            nc.vector.memset(w_nm[0:112, 3, :], 0.0)

            # ------------------------- pooling + classifier --------------------
            pooled_ps = pas.tile([64, gc], F32, tag="as")
            for g in range(gc):
                for c in range(4):
                    nc.tensor.matmul(pooled_ps[:, g:g + 1],
                                     h3keep[g][:, c, :],
                                     w_nm[:, c, g:g + 1],
                                     start=(c == 0), stop=(c == 3))
            pool_fm = cpool.tile([65, gc], B16, tag="poolfm")
            nc.vector.memset(pool_fm[64:65, :], 1.0)
            nc.scalar.activation(pool_fm[0:64, :], pooled_ps[:], AF.Copy,
                                 scale=1.0 / float(KTOP))
            if dbg:
                pfm_f = cpool.tile([65, gc], F32, tag="pfmf")
                nc.vector.tensor_copy(pfm_f[:], pool_fm[:])
                nc.sync.dma_start(out=dbg_pool[:], in_=pfm_f[:])
            plw = pas.tile([1, 2 * gc], F32, tag="as")
            for cls in range(2):
                nc.tensor.matmul(plw[0:1, cls * gc:(cls + 1) * gc],
                                 wlin[:, cls:cls + 1], pool_fm[:],
                                 start=True, stop=True)
            lgw = cpool.tile([1, 2 * gc], F32, tag="lgw")
            nc.vector.tensor_copy(lgw[:], plw[:])
            m01 = cpool.tile([1, gc], F32, tag="m01")
            d0 = cpool.tile([1, gc], F32, tag="d0")
            d1 = cpool.tile([1, gc], F32, tag="d1")
            e0 = cpool.tile([1, gc], F32, tag="e0")
            e1 = cpool.tile([1, gc], F32, tag="e1")
            lse = cpool.tile([1, gc], F32, tag="lse")
            out_sb = cpool.tile([1, 2 * gc], F32, tag="outsb")
            nc.vector.tensor_tensor(m01[:], lgw[:, 0:gc], lgw[:, gc:2 * gc],
                                    OP.max)
            nc.vector.tensor_tensor(d0[:], lgw[:, 0:gc], m01[:], OP.subtract)
            nc.vector.tensor_tensor(d1[:], lgw[:, gc:2 * gc], m01[:],
                                    OP.subtract)
            nc.scalar.activation(e0[:], d0[:], AF.Exp)
            nc.scalar.activation(e1[:], d1[:], AF.Exp)
            nc.vector.tensor_tensor(lse[:], e0[:], e1[:], OP.add)
            nc.scalar.activation(lse[:], lse[:], AF.Ln)
            nc.vector.tensor_tensor(out_sb[:, 0:gc], d0[:], lse[:], OP.subtract)
            nc.vector.tensor_tensor(out_sb[:, gc:2 * gc], d1[:], lse[:],
                                    OP.subtract)
            ov = out_sb[:].rearrange("p (a b) -> p a b", a=2)[:, :, 0:gc]
            nc.sync.dma_start(out=out_d[:], in_=ov)

    nc.compile()
    return nc


# ----------------------------------------------------------------------------
# Host-side shard/layout prep
# ----------------------------------------------------------------------------

def _prep(x, edge_index, W1l, W1r, b1, W2l, W2r, b2, W3l, W3r, b3,
          Wpr, bpr, Wpo, Wlin, blin, n_graphs=B):
    src = np.asarray(edge_index[0]).astype(np.int64) % NPG
    dst = np.asarray(edge_index[1]).astype(np.int64) % NPG
    key = (src * NPG + dst).reshape(n_graphs, EPG)

    A = np.zeros((n_graphs, NPG * NPG), np.float32)
    for g in range(n_graphs):
        A[g] = np.bincount(key[g], minlength=NPG * NPG)
    A = A.reshape(n_graphs, NPG, NPG)          # A[g, s, d] = edge count s->d
    deg = A.sum(axis=1)                        # in-degree per dst [g, 400]
    inv = 1.0 / np.maximum(deg, 1.0)
    An = A * inv[:, None, :]                   # column-normalized

    # overlapping src chunks (starts 0/128/256/272); chunk 3 keeps only
    # src rows 384:400 (its first 112 rows duplicate chunk 2 -> zeroed)
    adj = np.zeros((n_graphs, 4, 128, NPG), np.float32)
    for c in range(3):
        adj[:, c] = An[:, CS[c]:CS[c] + 128, :]
    adj[:, 3, 112:128, :] = An[:, 384:400, :]
    adj = np.ascontiguousarray(
        adj.transpose(0, 2, 1, 3).reshape(n_graphs, 128, 4 * NPG)).astype(BF16)

    # deg_nm[p, 4g + c] = deg[g, CS[c] + p]
    deg_nm = np.zeros((128, n_graphs, 4), np.float32)
    for c in range(4):
        deg_nm[:, :, c] = deg[:, CS[c]:CS[c] + 128].T
    deg_nm = np.ascontiguousarray(deg_nm.reshape(128, n_graphs * 4))

    x = np.asarray(x, np.float32)
    xT = x.reshape(n_graphs, NPG, F_IN).transpose(0, 2, 1)   # [g, 200, 400]
    xa = np.ascontiguousarray(xT[:, 0:128, :]).astype(BF16)
    xb = np.ascontiguousarray(xT[:, 128:200, :]).astype(BF16)

    def n_(a):
        return np.asarray(a, np.float32)

    cw16 = np.zeros((128, 520), np.float32)
    cw16[:, 0:128] = np.eye(128)
    w1cat = np.concatenate([n_(W1l), n_(W1r)], axis=1)       # [200, 128]
    cw16[:, 128:256] = w1cat[0:128]
    cw16[0:72, 256:384] = w1cat[128:200]
    cw16[:, 384:448] = np.concatenate([n_(W2r), n_(W2l)], axis=0)
    cw16[:, 448:512] = np.concatenate([n_(W3r), n_(W3l)], axis=0)
    cw16[0:64, 512:513] = n_(Wpo)
    cw16[64:128, 513:514] = n_(Wpr)
    cw16[0:64, 514:516] = n_(Wlin)
    cw16[64, 514:516] = n_(blin)
    cw16 = cw16.astype(BF16)

    cf32 = np.zeros((128, 136), np.float32)
    cf32[:, 0:128] = np.eye(128)
    cf32[64:128, 128] = n_(b1)     # biases read at partition base 64
    cf32[64:128, 129] = n_(b2)
    cf32[64:128, 130] = n_(b3)

    return xa, xb, adj, deg_nm, cw16, cf32, float(np.asarray(bpr).reshape(-1)[0])


def kernel(**inputs):
    x = inputs["x"]
    edge_index = inputs["edge_index"]
    xa, xb, adj, deg_nm, cw16, cf32, bpr_val = _prep(
        x, edge_index, inputs["W1l"], inputs["W1r"], inputs["b1"],
        inputs["W2l"], inputs["W2r"], inputs["b2"],
        inputs["W3l"], inputs["W3r"], inputs["b3"],
        inputs["Wpr"], inputs["bpr"], inputs["Wpo"],
        inputs["Wlin"], inputs["blin"])

    nc = build_kernel(G, bpr_val)

    in_maps = []
    for c in range(NCORES):
        gs = slice(c * G, (c + 1) * G)
        in_maps.append({
            "xa": np.ascontiguousarray(xa[gs]),
            "xb": np.ascontiguousarray(xb[gs]),
            "adj": np.ascontiguousarray(adj[gs]),
            "deg": np.ascontiguousarray(
                deg_nm[:, c * G * 4:(c + 1) * G * 4]),
            "cw16": cw16,
            "cf32": cf32,
        })
    res = run_bass_kernel_spmd(nc, in_maps, list(range(NCORES)))
    outs = [res.results[i]["out"] for i in range(NCORES)]    # each [2, G]
    logits = np.concatenate(outs, axis=1).T                  # [512, 2]
    return np.ascontiguousarray(logits.astype(np.float32))
